# revision 1
# baseline (speedup 1.0000x reference)
# Trainium2 Bass kernel for nn_Decoder_26087631356046 (ConvS2S-style decoder).
#
# Data-parallel over batch (B=32) across 8 NeuronCores; each core runs 4 rows
# as 2 passes of 2. Activations are channel-major [C, tokens] on-chip; the
# causal conv is 3 shifted matmuls with the ones-padding baked into the ci
# layout; softmax max-subtraction folds per-row maxes (from an l-major score
# pass) into the transposed-score PSUM group as a rank-1 (-1)*max matmul.
#
# Precision: the softmax here is extremely sharp (|scores| up to ~150), so any
# rounding upstream of it (conv, projections, QK) gets amplified into
# attention-weight flips. All matmuls on that path therefore run as bf16x2
# split-float (hi/lo) 3-term products — full PE rate with ~1e-5 relative
# error — with fp32 carried between layers.
import sys

if "/opt/trn_rl_repo" not in sys.path:
    sys.path.append("/opt/trn_rl_repo")

import numpy as np
import ml_dtypes

import concourse.bass as bass
import concourse.tile as tile
from concourse import bacc, mybir
from concourse.bass import ts
from concourse.bass_utils import run_bass_kernel_spmd
from concourse.masks import make_identity

F32 = mybir.dt.float32
R32 = mybir.dt.float32r
BF16 = mybir.dt.bfloat16
AF = mybir.ActivationFunctionType
ALU = mybir.AluOpType

B, L, X, H = 32, 512, 64, 512
Hh, H2, H4 = H // 2, H * 2, H * 4
NL = 5
NCORES = 8
B_LOC = B // NCORES
B_SUB = 2
NPASS = B_LOC // B_SUB
N = B_SUB * L


def build_nc(taps=False):
    nc = bacc.Bacc(trn_type="TRN2", target_bir_lowering=False, debug=False)

    def din(name, shape, dt=F32):
        return nc.dram_tensor(name, list(shape), dt, kind="ExternalInput").ap()

    target = din("target", (B_LOC, L, X))
    he = din("hidden_encoder", (B_LOC, L, Hh))
    re_hi = din("re_hi", (B_LOC, L, Hh), BF16)
    re_lo = din("re_lo", (B_LOC, L, Hh), BF16)
    lin_hi = din("lin_hi", (X, Hh), BF16)
    lin_lo = din("lin_lo", (X, Hh), BF16)
    lin_b = din("lin_b", (Hh,))
    pos_wT = din("pos_wT", (L, Hh))
    pos_b = din("pos_b", (Hh,))
    th_hi = din("th_hi", (Hh, H2), BF16)
    th_lo = din("th_lo", (Hh, H2), BF16)
    th_b = din("to_hidden_b", (H2,))
    cw_hi = din("cw_hi", (NL, 8, 8, 128, 2, 3, 128), BF16)
    cw_lo = din("cw_lo", (NL, 8, 8, 128, 2, 3, 128), BF16)
    conv_b = din("conv_b", (NL, H4))
    af_hi = din("af_hi", (H2, Hh), BF16)
    af_lo = din("af_lo", (H2, Hh), BF16)
    af_b = din("att_from_b", (Hh,))
    at_hi = din("at_hi", (Hh, H2), BF16)
    at_lo = din("at_lo", (Hh, H2), BF16)
    at_b = din("att_to_b", (H2,))
    fh_hi = din("fh_hi", (H2, Hh), BF16)
    fh_lo = din("fh_lo", (H2, Hh), BF16)
    fh_b = din("from_hidden_b", (Hh,))
    ow_hi = din("ow_hi", (Hh, X), BF16)
    ow_lo = din("ow_lo", (Hh, X), BF16)
    out_b = din("out_b", (X,))
    out = nc.dram_tensor("out", [B_LOC, L, X], F32, kind="ExternalOutput").ap()

    tap_t = {}
    if taps:
        for nm, shp in [("tap_h", (8, 128, B_SUB, L)),
                        ("tap_ci_l0", (8, 128, B_SUB, L))]:
            tap_t[nm] = nc.dram_tensor(nm, list(shp), mybir.dt.bfloat16,
                                       kind="ExternalOutput").ap()

    from contextlib import ExitStack

    with tile.TileContext(nc) as tc, ExitStack() as stack:
        persist = stack.enter_context(tc.tile_pool(name="persist", bufs=1))

        def split(hi, lo, src):
            """hi = bf16(src); lo = bf16(src - hi). src may be PSUM or SBUF."""
            nc.vector.tensor_copy(hi, src)
            nc.vector.tensor_sub(lo, src, hi)

        # --- constants ---------------------------------------------------
        ident = persist.tile([128, 128], F32, tag="ident", name="ident")
        make_identity(nc, ident)
        ones_bf = persist.tile([128, 1], BF16, tag="ones_bf", name="ones_bf")
        nc.vector.memset(ones_bf, 1.0)
        negones = persist.tile([1, 128], BF16, tag="negones", name="negones")
        nc.vector.memset(negones, -1.0)

        sb_conv_b = persist.tile([128, NL, 16], F32, tag="sb_conv_b",
                                 name="sb_conv_b")
        nc.sync.dma_start(sb_conv_b, conv_b.rearrange("l (t p) -> p l t", p=128))
        sb_th_b = persist.tile([128, 8], F32, tag="sb_th_b", name="sb_th_b")
        nc.sync.dma_start(sb_th_b, th_b.rearrange("(t p) -> p t", p=128))
        sb_af_b = persist.tile([128, 2], F32, tag="sb_af_b", name="sb_af_b")
        nc.sync.dma_start(sb_af_b, af_b.rearrange("(t p) -> p t", p=128))
        sb_at_b = persist.tile([128, 8], F32, tag="sb_at_b", name="sb_at_b")
        nc.sync.dma_start(sb_at_b, at_b.rearrange("(t p) -> p t", p=128))
        sb_fh_b = persist.tile([128, 2], F32, tag="sb_fh_b", name="sb_fh_b")
        nc.sync.dma_start(sb_fh_b, fh_b.rearrange("(t p) -> p t", p=128))
        sb_out_b = persist.tile([64, 1], F32, tag="sb_out_b", name="sb_out_b")
        nc.sync.dma_start(sb_out_b, out_b.rearrange("(t p) -> p t", p=64))
        sb_lin_b = persist.tile([128, 2], F32, tag="sb_lin_b", name="sb_lin_b")
        nc.sync.dma_start(sb_lin_b, lin_b.rearrange("(t p) -> p t", p=128))
        sb_pos_b = persist.tile([128, 2], F32, tag="sb_pos_b", name="sb_pos_b")
        nc.sync.dma_start(sb_pos_b, pos_b.rearrange("(t p) -> p t", p=128))

        def load_pair(hid, lod, shape, nm, pool=persist):
            thi = pool.tile(shape, BF16, tag=f"{nm}h", name=f"{nm}h")
            tlo = pool.tile(shape, BF16, tag=f"{nm}l", name=f"{nm}l")
            nc.sync.dma_start(thi, hid)
            nc.sync.dma_start(tlo, lod)
            return thi, tlo

        sb_lin = load_pair(lin_hi, lin_lo, [64, Hh], "lin")
        sb_af = [load_pair(af_hi[ts(i, 128), :], af_lo[ts(i, 128), :],
                           [128, Hh], f"af{i}") for i in range(8)]
        sb_at = [load_pair(at_hi[ts(i, 128), :], at_lo[ts(i, 128), :],
                           [128, H2], f"at{i}") for i in range(2)]
        sb_ow = [load_pair(ow_hi[ts(i, 128), :], ow_lo[ts(i, 128), :],
                           [128, X], f"ow{i}") for i in range(2)]

        def mm3(psum, lhs_pair, rhs_pair, start, stop):
            """3-term split-float matmul accumulate: hh + hl + lh."""
            lh, ll = lhs_pair
            rh, rl = rhs_pair
            nc.tensor.matmul(psum, lh, rh, start=start, stop=False)
            nc.tensor.matmul(psum, lh, rl, start=False, stop=False)
            nc.tensor.matmul(psum, ll, rh, start=False, stop=stop)

        # --- embbias = pos + pos_b + lin_b (exact fp32 matmul) -----------
        embbias = persist.tile([128, 2], F32, tag="embbias", name="embbias")
        with tc.tile_pool(name="posp", bufs=1) as posp, \
             tc.tile_pool(name="pos_psum", bufs=1, space="PSUM") as pos_psum:
            sb_pos_wT = [posp.tile([128, Hh], F32, tag=f"pw{lt}", name=f"pw{lt}")
                         for lt in range(4)]
            for lt in range(4):
                nc.sync.dma_start(sb_pos_wT[lt], pos_wT[ts(lt, 128), :])
            iota_i = posp.tile([128, 1], mybir.dt.int32, tag="iota_i",
                               name="iota_i")
            iota_f = [posp.tile([128, 1], F32, tag=f"iota_f{lt}",
                                name=f"iota_f{lt}") for lt in range(4)]
            for lt in range(4):
                nc.gpsimd.iota(iota_i, pattern=[[1, 1]], base=lt * 128,
                               channel_multiplier=1)
                nc.vector.tensor_copy(iota_f[lt], iota_i)
            for dt_ in range(2):
                pp = pos_psum.tile([128, 1], F32, tag="pp", name="pp")
                for lt in range(4):
                    nc.tensor.matmul(pp, sb_pos_wT[lt][:, ts(dt_, 128)],
                                     iota_f[lt], start=(lt == 0), stop=(lt == 3))
                nc.vector.scalar_tensor_tensor(
                    out=embbias[:, dt_:dt_ + 1], in0=pp,
                    scalar=sb_pos_b[:, dt_:dt_ + 1],
                    in1=sb_lin_b[:, dt_:dt_ + 1], op0=ALU.add, op1=ALU.add)

        # --- per-pass state (hi/lo bf16 pairs) ---------------------------
        embT = [persist.tile([128, N], F32, tag=f"embT{i}", name=f"embT{i}")
                for i in range(2)]
        heT = [[persist.tile([128, N], BF16, tag=f"heT{i}{s}", name=f"heT{i}{s}")
                for s in "hl"] for i in range(2)]
        sb_re = [[persist.tile([128, Hh], BF16, tag=f"re{i}{s}", name=f"re{i}{s}")
                  for s in "hl"] for i in range(8)]
        ci = [[persist.tile([128, B_SUB, L + 2], BF16, tag=f"ci{i}{s}",
                            name=f"ci{i}{s}") for s in "hl"] for i in range(8)]
        h = [[persist.tile([128, B_SUB, L], BF16, tag=f"h{i}{s}",
                           name=f"h{i}{s}") for s in "hl"] for i in range(8)]

        for p in range(NPASS):
            rows = [B_SUB * p + b for b in range(B_SUB)]

            # ===== init ==================================================
            with tc.tile_pool(name=f"init{p}", bufs=1) as initp, \
                 tc.tile_pool(name=f"initpm{p}", bufs=2, space="PSUM") as initpm:
                # targetT [64, N] fp32 via PE transpose, then split
                targetT = initp.tile([64, N], F32, tag="targetT", name="targetT")
                for t in range(8):
                    b, lt = divmod(t, 4)
                    tt = initp.tile([128, X], F32, tag="tt", name="tt", bufs=2)
                    nc.sync.dma_start(tt, target[rows[b], ts(lt, 128), :])
                    ptr = initpm.tile([64, 128], F32, tag="ptr", name="ptr")
                    nc.tensor.transpose(ptr, tt, ident)
                    nc.vector.tensor_copy(targetT[:, ts(t, 128)], ptr)
                tgt_hi = initp.tile([64, N], BF16, tag="tgt_hi", name="tgt_hi")
                tgt_lo = initp.tile([64, N], BF16, tag="tgt_lo", name="tgt_lo")
                split(tgt_hi, tgt_lo, targetT)
                # embT = lin.T @ targetT + embbias   (b3)
                for dt_ in range(2):
                    for nt in range(2):
                        pe_ = initpm.tile([128, 512], F32, tag="pe", name="pe")
                        mm3(pe_, (sb_lin[0][:, ts(dt_, 128)],
                                  sb_lin[1][:, ts(dt_, 128)]),
                            (tgt_hi[:, ts(nt, 512)], tgt_lo[:, ts(nt, 512)]),
                            True, True)
                        nc.vector.tensor_scalar_add(embT[dt_][:, ts(nt, 512)],
                                                    pe_, embbias[:, dt_:dt_ + 1])
                emb_hi = [initp.tile([128, N], BF16, tag=f"ebh{i}",
                                     name=f"ebh{i}") for i in range(2)]
                emb_lo = [initp.tile([128, N], BF16, tag=f"ebl{i}",
                                     name=f"ebl{i}") for i in range(2)]
                for dt_ in range(2):
                    split(emb_hi[dt_], emb_lo[dt_], embT[dt_])
                # heT via PE transpose -> split from psum; re from host split
                for t in range(8):
                    b, mt = divmod(t, 4)
                    het = initp.tile([128, Hh], F32, tag="het", name="het",
                                     bufs=2)
                    nc.sync.dma_start(het, he[rows[b], ts(mt, 128), :])
                    nc.sync.dma_start(sb_re[t][0], re_hi[rows[b], ts(mt, 128), :])
                    nc.sync.dma_start(sb_re[t][1], re_lo[rows[b], ts(mt, 128), :])
                    for dt_ in range(2):
                        pht = initpm.tile([128, 128], F32, tag="pht", name="pht")
                        nc.tensor.transpose(pht, het[:, ts(dt_, 128)], ident)
                        split(heT[dt_][0][:, ts(t, 128)],
                              heT[dt_][1][:, ts(t, 128)], pht)
                # ci = to_hidden(emb) + b  (b3); pads hi=1, lo=0
                sb_th = [load_pair(th_hi[ts(i, 128), :], th_lo[ts(i, 128), :],
                                   [128, H2], f"th{i}", pool=initp)
                         for i in range(2)]
                for c_t in range(8):
                    nc.vector.memset(ci[c_t][0][:, :, 0:2], 1.0)
                    nc.vector.memset(ci[c_t][1][:, :, 0:2], 0.0)
                    for b in range(B_SUB):
                        pc = initpm.tile([128, 512], F32, tag="pe", name="pe")
                        mm3(pc, (sb_th[0][0][:, ts(c_t, 128)],
                                 sb_th[0][1][:, ts(c_t, 128)]),
                            (emb_hi[0][:, ts(b, 512)], emb_lo[0][:, ts(b, 512)]),
                            True, False)
                        mm3(pc, (sb_th[1][0][:, ts(c_t, 128)],
                                 sb_th[1][1][:, ts(c_t, 128)]),
                            (emb_hi[1][:, ts(b, 512)], emb_lo[1][:, ts(b, 512)]),
                            False, True)
                        tmpci = initp.tile([128, 512], F32, tag="tmpci",
                                           name="tmpci", bufs=3)
                        nc.vector.tensor_scalar_add(tmpci, pc,
                                                    sb_th_b[:, c_t:c_t + 1])
                        split(ci[c_t][0][:, b, 2:], ci[c_t][1][:, b, 2:], tmpci)

            # ===== layers ===============================================
            with tc.tile_pool(name=f"convw{p}", bufs=10) as convp, \
                 tc.tile_pool(name=f"scratch{p}", bufs=1) as scr, \
                 tc.tile_pool(name=f"pmm{p}", bufs=2, space="PSUM") as pmm, \
                 tc.tile_pool(name=f"psm{p}", bufs=1, space="PSUM") as psm:
                for layer in range(NL):
                    # ---- conv + GLU (b3) ----
                    for pair in range(8):
                        wts = []
                        for i_t in range(8):
                            wh = convp.tile([128, 2, 3, 128], BF16, tag="cwh",
                                            name="cwh")
                            wl = convp.tile([128, 2, 3, 128], BF16, tag="cwl",
                                            name="cwl")
                            nc.sync.dma_start(wh, cw_hi[layer, pair, i_t])
                            nc.sync.dma_start(wl, cw_lo[layer, pair, i_t])
                            wts.append((wh, wl))
                        for b in range(B_SUB):
                            pa = pmm.tile([128, 512], F32, tag="pa", name="pa")
                            pb = pmm.tile([128, 512], F32, tag="pb", name="pb")
                            for i_t in range(8):
                                for k in range(3):
                                    rhs = (ci[i_t][0][:, b, k:k + 512],
                                           ci[i_t][1][:, b, k:k + 512])
                                    first = (i_t == 0 and k == 0)
                                    last = (i_t == 7 and k == 2)
                                    mm3(pa, (wts[i_t][0][:, 0, k, :],
                                             wts[i_t][1][:, 0, k, :]), rhs,
                                        first, last)
                                    mm3(pb, (wts[i_t][0][:, 1, k, :],
                                             wts[i_t][1][:, 1, k, :]), rhs,
                                        first, last)
                            sigb = scr.tile([128, 512], F32, tag="tmpf",
                                            name="sigb", bufs=6)
                            nc.scalar.activation(
                                sigb, pb, AF.Sigmoid,
                                bias=sb_conv_b[:, layer, 8 + pair:9 + pair])
                            th_ = scr.tile([128, 512], F32, tag="tmpf",
                                           name="th_", bufs=6)
                            nc.vector.scalar_tensor_tensor(
                                out=th_, in0=pa,
                                scalar=sb_conv_b[:, layer, pair:pair + 1],
                                in1=sigb, op0=ALU.add, op1=ALU.mult)
                            split(h[pair][0][:, b, :], h[pair][1][:, b, :], th_)
                    if taps and p == 0 and layer == 0:
                        for pr in range(8):
                            nc.sync.dma_start(tap_t["tap_h"][pr], h[pr][0])
                    # ---- attention ----
                    rc_hi = [scr.tile([128, N], BF16, tag=f"rch{i}",
                                      name=f"rch{i}", bufs=1) for i in range(2)]
                    rc_lo = [scr.tile([128, N], BF16, tag=f"rcl{i}",
                                      name=f"rcl{i}", bufs=1) for i in range(2)]
                    for dt_ in range(2):
                        for b in range(B_SUB):
                            prc = pmm.tile([128, 512], F32, tag="patt",
                                           name="patt", bufs=3)
                            for c_t in range(8):
                                mm3(prc, (sb_af[c_t][0][:, ts(dt_, 128)],
                                          sb_af[c_t][1][:, ts(dt_, 128)]),
                                    (h[c_t][0][:, b, :], h[c_t][1][:, b, :]),
                                    c_t == 0, c_t == 7)
                            trc = scr.tile([128, 512], F32, tag="tmpf",
                                           name="trc", bufs=6)
                            nc.vector.scalar_tensor_tensor(
                                out=trc, in0=prc,
                                scalar=sb_af_b[:, dt_:dt_ + 1],
                                in1=embT[dt_][:, ts(b, 512)], op0=ALU.add,
                                op1=ALU.add)
                            split(rc_hi[dt_][:, ts(b, 512)],
                                  rc_lo[dt_][:, ts(b, 512)], trc)
                    for b in range(B_SUB):
                        # per-row -max via l-major scores
                        maxes = scr.tile([128, 4], F32, tag="maxes",
                                         name="maxes", bufs=2)
                        for lt in range(4):
                            plm = pmm.tile([128, 512], F32, tag="patt",
                                           name="patt", bufs=3)
                            sl_ = ts(b * 4 + lt, 128)
                            nc.tensor.matmul(plm, rc_hi[0][:, sl_],
                                             heT[0][0][:, ts(b, 512)],
                                             start=True, stop=False)
                            nc.tensor.matmul(plm, rc_hi[1][:, sl_],
                                             heT[1][0][:, ts(b, 512)],
                                             start=False, stop=True)
                            nc.vector.tensor_reduce(
                                out=maxes[:, lt:lt + 1], in_=plm,
                                axis=mybir.AxisListType.X, op=ALU.max)
                        mrow = scr.tile([1, 512], BF16, tag="mrow", name="mrow",
                                        bufs=2)
                        for lt in range(4):
                            pmx = psm.tile([1, 128], F32, tag="psum_sum", name="pmx",
                                           bufs=1)
                            nc.tensor.transpose(pmx, maxes[:, lt:lt + 1], ident)
                            nc.vector.tensor_copy(mrow[:, ts(lt, 128)], pmx)
                        # transposed scores + exp + split + sum
                        e_hi = [scr.tile([128, 512], BF16, tag=f"eh{m}",
                                         name=f"eh{m}", bufs=1)
                                for m in range(4)]
                        e_lo = [scr.tile([128, 512], BF16, tag=f"el{m}",
                                         name=f"el{m}", bufs=1)
                                for m in range(4)]
                        psum_sum = psm.tile([1, 512], F32, tag="psum_sum",
                                            name="psum_sum")
                        for m_t in range(4):
                            ps = pmm.tile([128, 512], F32, tag="patt",
                                          name="patt", bufs=3)
                            sl_ = ts(b * 4 + m_t, 128)
                            mm3(ps, (heT[0][0][:, sl_], heT[0][1][:, sl_]),
                                (rc_hi[0][:, ts(b, 512)],
                                 rc_lo[0][:, ts(b, 512)]), True, False)
                            mm3(ps, (heT[1][0][:, sl_], heT[1][1][:, sl_]),
                                (rc_hi[1][:, ts(b, 512)],
                                 rc_lo[1][:, ts(b, 512)]), False, False)
                            nc.tensor.matmul(ps, negones, mrow, start=False,
                                             stop=True)
                            te = scr.tile([128, 512], F32, tag="tmpf", name="te",
                                          bufs=6)
                            nc.scalar.activation(te, ps, AF.Exp)
                            split(e_hi[m_t], e_lo[m_t], te)
                            nc.tensor.matmul(psum_sum, ones_bf, e_hi[m_t],
                                             start=(m_t == 0), stop=False)
                            nc.tensor.matmul(psum_sum, ones_bf, e_lo[m_t],
                                             start=False, stop=(m_t == 3))
                        recip = scr.tile([1, 512], F32, tag="recip",
                                         name="recip", bufs=2)
                        nc.vector.reciprocal(recip, psum_sum)
                        bcast = scr.tile([128, 512], F32, tag="bcast",
                                         name="bcast", bufs=1)
                        nc.gpsimd.partition_broadcast(bcast, recip)
                        # PV (b3) -> scale -> split ae
                        ae_hi = scr.tile([128, 2, 512], BF16, tag="aeh",
                                         name="aeh", bufs=2)
                        ae_lo = scr.tile([128, 2, 512], BF16, tag="ael",
                                         name="ael", bufs=2)
                        for dt_ in range(2):
                            ppv = pmm.tile([128, 512], F32, tag="patt",
                                           name="patt", bufs=3)
                            for m_t in range(4):
                                mm3(ppv,
                                    (sb_re[b * 4 + m_t][0][:, ts(dt_, 128)],
                                     sb_re[b * 4 + m_t][1][:, ts(dt_, 128)]),
                                    (e_hi[m_t], e_lo[m_t]), m_t == 0, m_t == 3)
                            tae = scr.tile([128, 512], F32, tag="tmpf",
                                           name="tae", bufs=6)
                            nc.vector.tensor_mul(tae, ppv, bcast)
                            split(ae_hi[:, dt_, :], ae_lo[:, dt_, :], tae)
                        # att_to (b3) + residuals
                        for c_t in range(8):
                            pat = pmm.tile([128, 512], F32, tag="patt",
                                           name="patt", bufs=3)
                            mm3(pat, (sb_at[0][0][:, ts(c_t, 128)],
                                      sb_at[0][1][:, ts(c_t, 128)]),
                                (ae_hi[:, 0, :], ae_lo[:, 0, :]), True, False)
                            mm3(pat, (sb_at[1][0][:, ts(c_t, 128)],
                                      sb_at[1][1][:, ts(c_t, 128)]),
                                (ae_hi[:, 1, :], ae_lo[:, 1, :]), False, True)
                            tht = scr.tile([128, 512], F32, tag="tmpf",
                                           name="tht", bufs=6)
                            nc.vector.scalar_tensor_tensor(
                                out=tht, in0=pat,
                                scalar=sb_at_b[:, c_t:c_t + 1],
                                in1=h[c_t][0][:, b, :], op0=ALU.add,
                                op1=ALU.add)
                            nc.vector.tensor_add(tht, tht, h[c_t][1][:, b, :])
                            split(h[c_t][0][:, b, :], h[c_t][1][:, b, :], tht)
                            tct = scr.tile([128, 512], F32, tag="tmpf",
                                           name="tct", bufs=6)
                            nc.vector.tensor_add(tct, tht, ci[c_t][0][:, b, 2:])
                            nc.vector.tensor_add(tct, tct, ci[c_t][1][:, b, 2:])
                            split(ci[c_t][0][:, b, 2:], ci[c_t][1][:, b, 2:],
                                  tct)
                    if taps and p == 0 and layer == 0:
                        for c_t in range(8):
                            nc.sync.dma_start(tap_t["tap_ci_l0"][c_t],
                                              ci[c_t][0][:, :, 2:])

            # ===== final head (b3) ======================================
            with tc.tile_pool(name=f"fin{p}", bufs=1) as finp, \
                 tc.tile_pool(name=f"finpm{p}", bufs=2, space="PSUM") as finpm:
                sb_fh = [load_pair(fh_hi[ts(i, 128), :], fh_lo[ts(i, 128), :],
                                   [128, Hh], f"fh{i}", pool=finp)
                         for i in range(8)]
                hid_hi = [finp.tile([128, N], BF16, tag=f"hih{i}",
                                    name=f"hih{i}") for i in range(2)]
                hid_lo = [finp.tile([128, N], BF16, tag=f"hil{i}",
                                    name=f"hil{i}") for i in range(2)]
                for dt_ in range(2):
                    for b in range(B_SUB):
                        ph = finpm.tile([128, 512], F32, tag="ph", name="ph")
                        for c_t in range(8):
                            mm3(ph, (sb_fh[c_t][0][:, ts(dt_, 128)],
                                     sb_fh[c_t][1][:, ts(dt_, 128)]),
                                (ci[c_t][0][:, b, 2:], ci[c_t][1][:, b, 2:]),
                                c_t == 0, c_t == 7)
                        thd = finp.tile([128, 512], F32, tag="thd", name="thd",
                                        bufs=3)
                        nc.vector.tensor_scalar_add(thd, ph,
                                                    sb_fh_b[:, dt_:dt_ + 1])
                        split(hid_hi[dt_][:, ts(b, 512)],
                              hid_lo[dt_][:, ts(b, 512)], thd)
                outT = finp.tile([64, N], F32, tag="outT", name="outT")
                for b in range(B_SUB):
                    po = finpm.tile([64, 512], F32, tag="po", name="po")
                    for dt_ in range(2):
                        mm3(po, (sb_ow[dt_][0], sb_ow[dt_][1]),
                            (hid_hi[dt_][:, ts(b, 512)],
                             hid_lo[dt_][:, ts(b, 512)]), dt_ == 0, dt_ == 1)
                    nc.vector.tensor_scalar_add(outT[:, ts(b, 512)], po,
                                                sb_out_b)
                for t in range(8):
                    b, lt = divmod(t, 4)
                    pt = finpm.tile([128, 64], F32, tag="pt", name="pt")
                    nc.tensor.transpose(pt, outT[:, ts(t, 128)], ident[:64, :64])
                    opos = finp.tile([128, X], F32, tag="opos", name="opos",
                                     bufs=2)
                    nc.vector.tensor_copy(opos, pt)
                    nc.sync.dma_start(out[rows[b], ts(lt, 128), :], opos)

    nc.compile()
    return nc


_NC_CACHE = None


def _get_nc():
    global _NC_CACHE
    if _NC_CACHE is None:
        _NC_CACHE = build_nc()
    return _NC_CACHE


def _split_np(x):
    bf = ml_dtypes.bfloat16
    x = np.asarray(x, np.float32)
    hi = np.ascontiguousarray(x.astype(bf))
    lo = np.ascontiguousarray((x - hi.astype(np.float32)).astype(bf))
    return hi, lo


def host_prep(inputs):
    f = np.float32
    w = {}
    w["lin_hi"], w["lin_lo"] = _split_np(np.asarray(inputs["lin_w"]).T)
    w["pos_wT"] = np.ascontiguousarray(np.asarray(inputs["pos_w"]).T, f)
    w["th_hi"], w["th_lo"] = _split_np(np.asarray(inputs["to_hidden_w"]).T)
    w["af_hi"], w["af_lo"] = _split_np(np.asarray(inputs["att_from_w"]).T)
    w["at_hi"], w["at_lo"] = _split_np(np.asarray(inputs["att_to_w"]).T)
    w["fh_hi"], w["fh_lo"] = _split_np(np.asarray(inputs["from_hidden_w"]).T)
    w["ow_hi"], w["ow_lo"] = _split_np(np.asarray(inputs["out_w"]).T)
    cw = np.asarray(inputs["conv_w"], f).reshape(NL, 2, 8, 128, H2, 3)
    cw = cw.transpose(0, 2, 4, 1, 5, 3)          # [l, pair, i, ab, k, o_p]
    cw = np.ascontiguousarray(cw.reshape(NL, 8, 8, 128, 2, 3, 128))
    w["cw_hi"], w["cw_lo"] = _split_np(cw)
    for k in ("lin_b", "pos_b", "to_hidden_b", "conv_b", "att_from_b",
              "att_to_b", "from_hidden_b", "out_b"):
        w[k] = np.ascontiguousarray(inputs[k], f)
    return w


LAST_RES = None


def kernel(_trace=False, **inputs):
    global LAST_RES
    nc = _get_nc()
    w = host_prep(inputs)
    re_hi, re_lo = _split_np(np.asarray(inputs["residual_encoder"]))
    in_maps = []
    for c in range(NCORES):
        sl = slice(B_LOC * c, B_LOC * (c + 1))
        m = dict(w)
        m["target"] = np.ascontiguousarray(inputs["target"][sl], np.float32)
        m["hidden_encoder"] = np.ascontiguousarray(
            inputs["hidden_encoder"][sl], np.float32)
        m["re_hi"] = np.ascontiguousarray(re_hi[sl])
        m["re_lo"] = np.ascontiguousarray(re_lo[sl])
        in_maps.append(m)
    if _trace:
        try:
            import antenv.axon_hooks  # noqa: F401
        except ImportError:
            _trace = False
    res = run_bass_kernel_spmd(nc, in_maps, core_ids=list(range(NCORES)),
                               trace=_trace)
    LAST_RES = res
    return np.concatenate([res.results[c]["out"] for c in range(NCORES)],
                          axis=0)



# revision 4
# speedup vs baseline: 3.1697x; 3.1697x over previous
# Trainium2 Bass kernel for nn_Decoder_26087631356046 (ConvS2S-style decoder).
#
# Data-parallel over batch (B=32) across 8 NeuronCores; each core runs 4 rows
# as 2 passes of 2. Activations are channel-major [C, tokens] on-chip; the
# causal conv is 3 shifted matmuls with the ones-padding baked into the ci
# layout; softmax max-subtraction folds per-row maxes (from an l-major score
# pass) into the transposed-score PSUM group as a rank-1 (-1)*max matmul.
#
# Precision: the softmax is extremely sharp (|scores| up to ~150) and the
# network amplifies upstream rounding ~1000x, so matmuls carry ~fp32-level
# operands as fp16 hi/lo split pairs. Attention/projection matmuls use the
# 3-term split product (hh + hl + lh) at full PE rate. The conv (84% of all
# MACs) instead computes hh as one fp16 matmul and BOTH cross terms in a
# single fp8e4m3 DoubleRow matmul (2 MACs/cell/cycle, 0.5 cycles/row): the
# cross terms are ~2^-12-scale corrections, so fp8 accuracy on them keeps
# the total product error at ~2^-15. Conv lo operands are pre-scaled by 2048
# (hi weights too, so one PSUM group accumulates 2048*full), descaled in the
# GLU epilogue via the ScalarE/DVE scale parameters.
import sys

if "/opt/trn_rl_repo" not in sys.path:
    sys.path.append("/opt/trn_rl_repo")

import numpy as np
import ml_dtypes

import concourse.bass as bass
import concourse.tile as tile
from concourse import bacc, mybir
from concourse.bass import ts
from concourse.bass_utils import run_bass_kernel_spmd
from concourse.masks import make_identity

F32 = mybir.dt.float32
F16 = mybir.dt.float16
F8 = mybir.dt.float8e4
E4 = ml_dtypes.float8_e4m3
AF = mybir.ActivationFunctionType
ALU = mybir.AluOpType
DR = mybir.MatmulPerfMode.DoubleRow

B, L, X, H = 32, 512, 64, 512
Hh, H2, H4 = H // 2, H * 2, H * 4
NL = 5
NCORES = 8
B_LOC = B // NCORES
B_SUB = 2
NPASS = B_LOC // B_SUB
N = B_SUB * L
CS = 2048.0      # conv lo-part / psum scale
CSI = 1.0 / CS
CIP = 520        # padded ci8 token stride (2*520 % 16 == 0 for DoubleRow AP)


def build_nc():
    nc = bacc.Bacc(trn_type="TRN2", target_bir_lowering=False, debug=False)

    def din(name, shape, dt=F32):
        return nc.dram_tensor(name, list(shape), dt, kind="ExternalInput").ap()

    target = din("target", (B_LOC, L, X))
    he = din("hidden_encoder", (B_LOC, L, Hh))
    re_hi = din("re_hi", (B_LOC, L, Hh), F16)
    re_lo = din("re_lo", (B_LOC, L, Hh), F16)
    lin_hi = din("lin_hi", (X, Hh), F16)
    lin_lo = din("lin_lo", (X, Hh), F16)
    lin_b = din("lin_b", (Hh,))
    pos_wT = din("pos_wT", (L, Hh))
    pos_b = din("pos_b", (Hh,))
    th_hi = din("th_hi", (Hh, H2), F16)
    th_lo = din("th_lo", (Hh, H2), F16)
    th_b = din("to_hidden_b", (H2,))
    cw_hi = din("cw_hi", (NL, 8, 8, 128, 2, 3, 128), F16)   # hi * CS
    cw8 = din("cw8", (NL, 8, 8, 128, 2, 3, 2, 128), F8)     # [lo*CS, hi]
    conv_b = din("conv_b", (NL, H4))
    af_hi = din("af_hi", (H2, Hh), F16)
    af_lo = din("af_lo", (H2, Hh), F16)
    af_b = din("att_from_b", (Hh,))
    at_hi = din("at_hi", (Hh, H2), F16)
    at_lo = din("at_lo", (Hh, H2), F16)
    at_b = din("att_to_b", (H2,))
    fh_hi = din("fh_hi", (H2, Hh), F16)
    fh_lo = din("fh_lo", (H2, Hh), F16)
    fh_b = din("from_hidden_b", (Hh,))
    ow_hi = din("ow_hi", (Hh, X), F16)
    ow_lo = din("ow_lo", (Hh, X), F16)
    out_b = din("out_b", (X,))
    out = nc.dram_tensor("out", [B_LOC, L, X], F32, kind="ExternalOutput").ap()

    from contextlib import ExitStack

    with tile.TileContext(nc) as tc, ExitStack() as stack:
        persist = stack.enter_context(tc.tile_pool(name="persist", bufs=1))

        def split(hi, lo, src):
            """hi = f16(src); lo = f16(src - hi). src may be PSUM or SBUF."""
            nc.vector.tensor_copy(hi, src)
            nc.vector.tensor_sub(lo, src, hi)

        # --- constants ---------------------------------------------------
        ident = persist.tile([128, 128], F32, tag="ident", name="ident")
        make_identity(nc, ident)
        ones_f = persist.tile([128, 1], F16, tag="ones_f", name="ones_f")
        nc.vector.memset(ones_f, 1.0)
        negones = persist.tile([1, 128], F16, tag="negones", name="negones")
        nc.vector.memset(negones, -1.0)

        sb_conv_b = persist.tile([128, NL, 16], F32, tag="sb_conv_b",
                                 name="sb_conv_b")
        nc.sync.dma_start(sb_conv_b, conv_b.rearrange("l (t p) -> p l t", p=128))
        sb_th_b = persist.tile([128, 8], F32, tag="sb_th_b", name="sb_th_b")
        nc.sync.dma_start(sb_th_b, th_b.rearrange("(t p) -> p t", p=128))
        sb_af_b = persist.tile([128, 2], F32, tag="sb_af_b", name="sb_af_b")
        nc.sync.dma_start(sb_af_b, af_b.rearrange("(t p) -> p t", p=128))
        sb_at_b = persist.tile([128, 8], F32, tag="sb_at_b", name="sb_at_b")
        nc.sync.dma_start(sb_at_b, at_b.rearrange("(t p) -> p t", p=128))
        sb_fh_b = persist.tile([128, 2], F32, tag="sb_fh_b", name="sb_fh_b")
        nc.sync.dma_start(sb_fh_b, fh_b.rearrange("(t p) -> p t", p=128))
        sb_out_b = persist.tile([64, 1], F32, tag="sb_out_b", name="sb_out_b")
        nc.sync.dma_start(sb_out_b, out_b.rearrange("(t p) -> p t", p=64))
        sb_lin_b = persist.tile([128, 2], F32, tag="sb_lin_b", name="sb_lin_b")
        nc.sync.dma_start(sb_lin_b, lin_b.rearrange("(t p) -> p t", p=128))
        sb_pos_b = persist.tile([128, 2], F32, tag="sb_pos_b", name="sb_pos_b")
        nc.sync.dma_start(sb_pos_b, pos_b.rearrange("(t p) -> p t", p=128))

        def load_pair(hid, lod, shape, nm, pool=persist):
            thi = pool.tile(shape, F16, tag=f"{nm}h", name=f"{nm}h")
            tlo = pool.tile(shape, F16, tag=f"{nm}l", name=f"{nm}l")
            nc.sync.dma_start(thi, hid)
            nc.sync.dma_start(tlo, lod)
            return thi, tlo

        sb_lin = load_pair(lin_hi, lin_lo, [64, Hh], "lin")
        sb_af = [load_pair(af_hi[ts(i, 128), :], af_lo[ts(i, 128), :],
                           [128, Hh], f"af{i}") for i in range(8)]
        sb_at = [load_pair(at_hi[ts(i, 128), :], at_lo[ts(i, 128), :],
                           [128, H2], f"at{i}") for i in range(2)]
        sb_ow = [load_pair(ow_hi[ts(i, 128), :], ow_lo[ts(i, 128), :],
                           [128, X], f"ow{i}") for i in range(2)]

        def mm3(psum, lhs_pair, rhs_pair, start, stop):
            """3-term split-float matmul accumulate: hh + hl + lh."""
            lh, ll = lhs_pair
            rh, rl = rhs_pair
            nc.tensor.matmul(psum, lh, rh, start=start, stop=False)
            nc.tensor.matmul(psum, lh, rl, start=False, stop=False)
            nc.tensor.matmul(psum, ll, rh, start=False, stop=stop)

        # --- embbias = pos + pos_b + lin_b (exact fp32 matmul) -----------
        embbias = persist.tile([128, 2], F32, tag="embbias", name="embbias")
        with tc.tile_pool(name="posp", bufs=1) as posp, \
             tc.tile_pool(name="pos_psum", bufs=1, space="PSUM") as pos_psum:
            sb_pos_wT = [posp.tile([128, Hh], F32, tag=f"pw{lt}", name=f"pw{lt}")
                         for lt in range(4)]
            for lt in range(4):
                nc.sync.dma_start(sb_pos_wT[lt], pos_wT[ts(lt, 128), :])
            iota_i = posp.tile([128, 1], mybir.dt.int32, tag="iota_i",
                               name="iota_i")
            iota_f = [posp.tile([128, 1], F32, tag=f"iota_f{lt}",
                                name=f"iota_f{lt}") for lt in range(4)]
            for lt in range(4):
                nc.gpsimd.iota(iota_i, pattern=[[1, 1]], base=lt * 128,
                               channel_multiplier=1)
                nc.vector.tensor_copy(iota_f[lt], iota_i)
            for dt_ in range(2):
                pp = pos_psum.tile([128, 1], F32, tag="pp", name="pp")
                for lt in range(4):
                    nc.tensor.matmul(pp, sb_pos_wT[lt][:, ts(dt_, 128)],
                                     iota_f[lt], start=(lt == 0), stop=(lt == 3))
                nc.vector.scalar_tensor_tensor(
                    out=embbias[:, dt_:dt_ + 1], in0=pp,
                    scalar=sb_pos_b[:, dt_:dt_ + 1],
                    in1=sb_lin_b[:, dt_:dt_ + 1], op0=ALU.add, op1=ALU.add)

        # --- per-pass state ---------------------------------------------
        embT = [persist.tile([128, N], F32, tag=f"embT{i}", name=f"embT{i}")
                for i in range(2)]
        heT = [[persist.tile([128, N], F16, tag=f"heT{i}{s}", name=f"heT{i}{s}")
                for s in "hl"] for i in range(2)]
        sb_re = [[persist.tile([128, Hh], F16, tag=f"re{i}{s}", name=f"re{i}{s}")
                  for s in "hl"] for i in range(8)]
        ci = [[persist.tile([128, B_SUB, L + 2], F16, tag=f"ci{i}{s}",
                            name=f"ci{i}{s}") for s in "hl"] for i in range(8)]
        ci8 = [persist.tile([128, 2, B_SUB, CIP], F8, tag=f"ci8_{i}",
                            name=f"ci8_{i}") for i in range(8)]
        h = [[persist.tile([128, B_SUB, L], F16, tag=f"h{i}{s}",
                           name=f"h{i}{s}") for s in "hl"] for i in range(8)]

        def ci_store(c_t, b, src):
            """split src (f32) into ci pairs + fp8 DoubleRow operand tiles."""
            split(ci[c_t][0][:, b, 2:], ci[c_t][1][:, b, 2:], src)
            nc.vector.tensor_copy(ci8[c_t][:, 0, b, 2:2 + L], src)
            nc.scalar.mul(ci8[c_t][:, 1, b, 2:2 + L],
                          ci[c_t][1][:, b, 2:], CS)

        for p in range(NPASS):
            rows = [B_SUB * p + b for b in range(B_SUB)]

            # ===== init ==================================================
            with tc.tile_pool(name=f"init{p}", bufs=1) as initp, \
                 tc.tile_pool(name=f"initpm{p}", bufs=2, space="PSUM") as initpm:
                # targetT [64, N] fp32 via PE transpose, then split
                targetT = initp.tile([64, N], F32, tag="targetT", name="targetT")
                for t in range(8):
                    b, lt = divmod(t, 4)
                    tt = initp.tile([128, X], F32, tag="tt", name="tt", bufs=2)
                    nc.sync.dma_start(tt, target[rows[b], ts(lt, 128), :])
                    ptr = initpm.tile([64, 128], F32, tag="ptr", name="ptr")
                    nc.tensor.transpose(ptr, tt, ident)
                    nc.vector.tensor_copy(targetT[:, ts(t, 128)], ptr)
                tgt_hi = initp.tile([64, N], F16, tag="tgt_hi", name="tgt_hi")
                tgt_lo = initp.tile([64, N], F16, tag="tgt_lo", name="tgt_lo")
                split(tgt_hi, tgt_lo, targetT)
                # embT = lin.T @ targetT + embbias
                for dt_ in range(2):
                    for nt in range(2):
                        pe_ = initpm.tile([128, 512], F32, tag="pe", name="pe")
                        mm3(pe_, (sb_lin[0][:, ts(dt_, 128)],
                                  sb_lin[1][:, ts(dt_, 128)]),
                            (tgt_hi[:, ts(nt, 512)], tgt_lo[:, ts(nt, 512)]),
                            True, True)
                        nc.vector.tensor_scalar_add(embT[dt_][:, ts(nt, 512)],
                                                    pe_, embbias[:, dt_:dt_ + 1])
                emb_hi = [initp.tile([128, N], F16, tag=f"ebh{i}",
                                     name=f"ebh{i}") for i in range(2)]
                emb_lo = [initp.tile([128, N], F16, tag=f"ebl{i}",
                                     name=f"ebl{i}") for i in range(2)]
                for dt_ in range(2):
                    split(emb_hi[dt_], emb_lo[dt_], embT[dt_])
                # heT via PE transpose -> split from psum; re from host split
                for t in range(8):
                    b, mt = divmod(t, 4)
                    het = initp.tile([128, Hh], F32, tag="het", name="het",
                                     bufs=2)
                    nc.sync.dma_start(het, he[rows[b], ts(mt, 128), :])
                    nc.sync.dma_start(sb_re[t][0], re_hi[rows[b], ts(mt, 128), :])
                    nc.sync.dma_start(sb_re[t][1], re_lo[rows[b], ts(mt, 128), :])
                    for dt_ in range(2):
                        pht = initpm.tile([128, 128], F32, tag="pht", name="pht")
                        nc.tensor.transpose(pht, het[:, ts(dt_, 128)], ident)
                        split(heT[dt_][0][:, ts(t, 128)],
                              heT[dt_][1][:, ts(t, 128)], pht)
                # ci = to_hidden(emb) + b; pads hi=1, lo=0 (fp8: [lo*CS, hi])
                sb_th = [load_pair(th_hi[ts(i, 128), :], th_lo[ts(i, 128), :],
                                   [128, H2], f"th{i}", pool=initp)
                         for i in range(2)]
                for c_t in range(8):
                    nc.vector.memset(ci[c_t][0][:, :, 0:2], 1.0)
                    nc.vector.memset(ci[c_t][1][:, :, 0:2], 0.0)
                    nc.vector.memset(ci8[c_t][:, 0, :, 0:2], 1.0)
                    nc.vector.memset(ci8[c_t][:, 1, :, 0:2], 0.0)
                    for b in range(B_SUB):
                        pc = initpm.tile([128, 512], F32, tag="pe", name="pe")
                        mm3(pc, (sb_th[0][0][:, ts(c_t, 128)],
                                 sb_th[0][1][:, ts(c_t, 128)]),
                            (emb_hi[0][:, ts(b, 512)], emb_lo[0][:, ts(b, 512)]),
                            True, False)
                        mm3(pc, (sb_th[1][0][:, ts(c_t, 128)],
                                 sb_th[1][1][:, ts(c_t, 128)]),
                            (emb_hi[1][:, ts(b, 512)], emb_lo[1][:, ts(b, 512)]),
                            False, True)
                        tmpci = initp.tile([128, 512], F32, tag="tmpci",
                                           name="tmpci", bufs=3)
                        nc.vector.tensor_scalar_add(tmpci, pc,
                                                    sb_th_b[:, c_t:c_t + 1])
                        ci_store(c_t, b, tmpci)

            # ===== layers ===============================================
            with tc.tile_pool(name=f"convw{p}", bufs=10) as convp, \
                 tc.tile_pool(name=f"scratch{p}", bufs=1) as scr, \
                 tc.tile_pool(name=f"pmm{p}", bufs=2, space="PSUM") as pmm, \
                 tc.tile_pool(name=f"psm{p}", bufs=1, space="PSUM") as psm:
                for layer in range(NL):
                    # ---- conv + GLU: f16 hh + fp8 DoubleRow cross ----
                    for pair in range(8):
                        wts, w8s = [], []
                        for i_t in range(8):
                            wh = convp.tile([128, 2, 3, 128], F16, tag="cwh",
                                            name="cwh")
                            w8 = convp.tile([128, 2, 3, 2, 128], F8, tag="cw8",
                                            name="cw8")
                            nc.sync.dma_start(wh, cw_hi[layer, pair, i_t])
                            nc.sync.dma_start(w8, cw8[layer, pair, i_t])
                            wts.append(wh)
                            w8s.append(w8)
                        for b in range(B_SUB):
                            pa = pmm.tile([128, 512], F32, tag="pa", name="pa")
                            pb = pmm.tile([128, 512], F32, tag="pb", name="pb")
                            for i_t in range(8):
                                for k in range(3):
                                    first = (i_t == 0 and k == 0)
                                    last = (i_t == 7 and k == 2)
                                    rhs_h = ci[i_t][0][:, b, k:k + 512]
                                    rhs_8 = ci8[i_t][:, :, b, k:k + 512]
                                    nc.tensor.matmul(
                                        pa, wts[i_t][:, 0, k, :], rhs_h,
                                        start=first, stop=False)
                                    nc.tensor.matmul(
                                        pa, w8s[i_t][:, 0, k, :, :], rhs_8,
                                        start=False, stop=last, perf_mode=DR)
                                    nc.tensor.matmul(
                                        pb, wts[i_t][:, 1, k, :], rhs_h,
                                        start=first, stop=False)
                                    nc.tensor.matmul(
                                        pb, w8s[i_t][:, 1, k, :, :], rhs_8,
                                        start=False, stop=last, perf_mode=DR)
                            sigb = scr.tile([128, 512], F32, tag="tmpf",
                                            name="sigb", bufs=6)
                            nc.scalar.activation(
                                sigb, pb, AF.Sigmoid,
                                bias=sb_conv_b[:, layer, 8 + pair:9 + pair],
                                scale=CSI)
                            ta = scr.tile([128, 512], F32, tag="tmpf",
                                          name="ta", bufs=6)
                            nc.vector.tensor_scalar(
                                out=ta, in0=pa, scalar1=CSI,
                                scalar2=sb_conv_b[:, layer, pair:pair + 1],
                                op0=ALU.mult, op1=ALU.add)
                            th_ = scr.tile([128, 512], F32, tag="tmpf",
                                           name="th_", bufs=6)
                            nc.vector.tensor_mul(th_, ta, sigb)
                            split(h[pair][0][:, b, :], h[pair][1][:, b, :], th_)
                    # ---- attention ----
                    rc_hi = [scr.tile([128, N], F16, tag=f"rch{i}",
                                      name=f"rch{i}", bufs=1) for i in range(2)]
                    rc_lo = [scr.tile([128, N], F16, tag=f"rcl{i}",
                                      name=f"rcl{i}", bufs=1) for i in range(2)]
                    for dt_ in range(2):
                        for b in range(B_SUB):
                            prc = pmm.tile([128, 512], F32, tag="patt",
                                           name="patt", bufs=3)
                            for c_t in range(8):
                                mm3(prc, (sb_af[c_t][0][:, ts(dt_, 128)],
                                          sb_af[c_t][1][:, ts(dt_, 128)]),
                                    (h[c_t][0][:, b, :], h[c_t][1][:, b, :]),
                                    c_t == 0, c_t == 7)
                            trc = scr.tile([128, 512], F32, tag="tmpf",
                                           name="trc", bufs=6)
                            nc.vector.scalar_tensor_tensor(
                                out=trc, in0=prc,
                                scalar=sb_af_b[:, dt_:dt_ + 1],
                                in1=embT[dt_][:, ts(b, 512)], op0=ALU.add,
                                op1=ALU.add)
                            split(rc_hi[dt_][:, ts(b, 512)],
                                  rc_lo[dt_][:, ts(b, 512)], trc)
                    for b in range(B_SUB):
                        # per-row -max via l-major scores
                        maxes = scr.tile([128, 4], F32, tag="maxes",
                                         name="maxes", bufs=2)
                        for lt in range(4):
                            plm = pmm.tile([128, 512], F32, tag="patt",
                                           name="patt", bufs=3)
                            sl_ = ts(b * 4 + lt, 128)
                            nc.tensor.matmul(plm, rc_hi[0][:, sl_],
                                             heT[0][0][:, ts(b, 512)],
                                             start=True, stop=False)
                            nc.tensor.matmul(plm, rc_hi[1][:, sl_],
                                             heT[1][0][:, ts(b, 512)],
                                             start=False, stop=True)
                            nc.vector.tensor_reduce(
                                out=maxes[:, lt:lt + 1], in_=plm,
                                axis=mybir.AxisListType.X, op=ALU.max)
                        mrow = scr.tile([1, 512], F16, tag="mrow", name="mrow",
                                        bufs=2)
                        for lt in range(4):
                            pmx = psm.tile([1, 128], F32, tag="psum_sum",
                                           name="pmx", bufs=1)
                            nc.tensor.transpose(pmx, maxes[:, lt:lt + 1], ident)
                            nc.vector.tensor_copy(mrow[:, ts(lt, 128)], pmx)
                        # transposed scores + exp + split + sum
                        e_hi = [scr.tile([128, 512], F16, tag=f"eh{m}",
                                         name=f"eh{m}", bufs=1)
                                for m in range(4)]
                        e_lo = [scr.tile([128, 512], F16, tag=f"el{m}",
                                         name=f"el{m}", bufs=1)
                                for m in range(4)]
                        psum_sum = psm.tile([1, 512], F32, tag="psum_sum",
                                            name="psum_sum")
                        for m_t in range(4):
                            ps = pmm.tile([128, 512], F32, tag="patt",
                                          name="patt", bufs=3)
                            sl_ = ts(b * 4 + m_t, 128)
                            mm3(ps, (heT[0][0][:, sl_], heT[0][1][:, sl_]),
                                (rc_hi[0][:, ts(b, 512)],
                                 rc_lo[0][:, ts(b, 512)]), True, False)
                            mm3(ps, (heT[1][0][:, sl_], heT[1][1][:, sl_]),
                                (rc_hi[1][:, ts(b, 512)],
                                 rc_lo[1][:, ts(b, 512)]), False, False)
                            nc.tensor.matmul(ps, negones, mrow, start=False,
                                             stop=True)
                            te = scr.tile([128, 512], F32, tag="tmpf", name="te",
                                          bufs=6)
                            nc.scalar.activation(te, ps, AF.Exp)
                            split(e_hi[m_t], e_lo[m_t], te)
                            nc.tensor.matmul(psum_sum, ones_f, e_hi[m_t],
                                             start=(m_t == 0), stop=False)
                            nc.tensor.matmul(psum_sum, ones_f, e_lo[m_t],
                                             start=False, stop=(m_t == 3))
                        recip = scr.tile([1, 512], F32, tag="recip",
                                         name="recip", bufs=2)
                        nc.vector.reciprocal(recip, psum_sum)
                        bcast = scr.tile([128, 512], F32, tag="bcast",
                                         name="bcast", bufs=1)
                        nc.gpsimd.partition_broadcast(bcast, recip)
                        # PV -> scale -> split ae
                        ae_hi = scr.tile([128, 2, 512], F16, tag="aeh",
                                         name="aeh", bufs=2)
                        ae_lo = scr.tile([128, 2, 512], F16, tag="ael",
                                         name="ael", bufs=2)
                        for dt_ in range(2):
                            ppv = pmm.tile([128, 512], F32, tag="patt",
                                           name="patt", bufs=3)
                            for m_t in range(4):
                                mm3(ppv,
                                    (sb_re[b * 4 + m_t][0][:, ts(dt_, 128)],
                                     sb_re[b * 4 + m_t][1][:, ts(dt_, 128)]),
                                    (e_hi[m_t], e_lo[m_t]), m_t == 0, m_t == 3)
                            tae = scr.tile([128, 512], F32, tag="tmpf",
                                           name="tae", bufs=6)
                            nc.vector.tensor_mul(tae, ppv, bcast)
                            split(ae_hi[:, dt_, :], ae_lo[:, dt_, :], tae)
                        # att_to + residuals
                        for c_t in range(8):
                            pat = pmm.tile([128, 512], F32, tag="patt",
                                           name="patt", bufs=3)
                            mm3(pat, (sb_at[0][0][:, ts(c_t, 128)],
                                      sb_at[0][1][:, ts(c_t, 128)]),
                                (ae_hi[:, 0, :], ae_lo[:, 0, :]), True, False)
                            mm3(pat, (sb_at[1][0][:, ts(c_t, 128)],
                                      sb_at[1][1][:, ts(c_t, 128)]),
                                (ae_hi[:, 1, :], ae_lo[:, 1, :]), False, True)
                            tht = scr.tile([128, 512], F32, tag="tmpf",
                                           name="tht", bufs=6)
                            nc.vector.scalar_tensor_tensor(
                                out=tht, in0=pat,
                                scalar=sb_at_b[:, c_t:c_t + 1],
                                in1=h[c_t][0][:, b, :], op0=ALU.add,
                                op1=ALU.add)
                            nc.vector.tensor_add(tht, tht, h[c_t][1][:, b, :])
                            split(h[c_t][0][:, b, :], h[c_t][1][:, b, :], tht)
                            tct = scr.tile([128, 512], F32, tag="tmpf",
                                           name="tct", bufs=6)
                            nc.vector.tensor_add(tct, tht, ci[c_t][0][:, b, 2:])
                            nc.vector.tensor_add(tct, tct, ci[c_t][1][:, b, 2:])
                            ci_store(c_t, b, tct)

            # ===== final head ===========================================
            with tc.tile_pool(name=f"fin{p}", bufs=1) as finp, \
                 tc.tile_pool(name=f"finpm{p}", bufs=2, space="PSUM") as finpm:
                sb_fh = [load_pair(fh_hi[ts(i, 128), :], fh_lo[ts(i, 128), :],
                                   [128, Hh], f"fh{i}", pool=finp)
                         for i in range(8)]
                hid_hi = [finp.tile([128, N], F16, tag=f"hih{i}",
                                    name=f"hih{i}") for i in range(2)]
                hid_lo = [finp.tile([128, N], F16, tag=f"hil{i}",
                                    name=f"hil{i}") for i in range(2)]
                for dt_ in range(2):
                    for b in range(B_SUB):
                        ph = finpm.tile([128, 512], F32, tag="ph", name="ph")
                        for c_t in range(8):
                            mm3(ph, (sb_fh[c_t][0][:, ts(dt_, 128)],
                                     sb_fh[c_t][1][:, ts(dt_, 128)]),
                                (ci[c_t][0][:, b, 2:], ci[c_t][1][:, b, 2:]),
                                c_t == 0, c_t == 7)
                        thd = finp.tile([128, 512], F32, tag="thd", name="thd",
                                        bufs=3)
                        nc.vector.tensor_scalar_add(thd, ph,
                                                    sb_fh_b[:, dt_:dt_ + 1])
                        split(hid_hi[dt_][:, ts(b, 512)],
                              hid_lo[dt_][:, ts(b, 512)], thd)
                outT = finp.tile([64, N], F32, tag="outT", name="outT")
                for b in range(B_SUB):
                    po = finpm.tile([64, 512], F32, tag="po", name="po")
                    for dt_ in range(2):
                        mm3(po, (sb_ow[dt_][0], sb_ow[dt_][1]),
                            (hid_hi[dt_][:, ts(b, 512)],
                             hid_lo[dt_][:, ts(b, 512)]), dt_ == 0, dt_ == 1)
                    nc.vector.tensor_scalar_add(outT[:, ts(b, 512)], po,
                                                sb_out_b)
                for t in range(8):
                    b, lt = divmod(t, 4)
                    pt = finpm.tile([128, 64], F32, tag="pt", name="pt")
                    nc.tensor.transpose(pt, outT[:, ts(t, 128)], ident[:64, :64])
                    opos = finp.tile([128, X], F32, tag="opos", name="opos",
                                     bufs=2)
                    nc.vector.tensor_copy(opos, pt)
                    nc.sync.dma_start(out[rows[b], ts(lt, 128), :], opos)

    nc.compile()
    return nc


_NC_CACHE = None


def _get_nc():
    global _NC_CACHE
    if _NC_CACHE is None:
        _NC_CACHE = build_nc()
    return _NC_CACHE


def _split16(x):
    x = np.asarray(x, np.float32)
    hi = np.ascontiguousarray(x.astype(np.float16))
    lo = np.ascontiguousarray((x - hi.astype(np.float32)).astype(np.float16))
    return hi, lo


def host_prep(inputs):
    f = np.float32
    w = {}
    w["lin_hi"], w["lin_lo"] = _split16(np.asarray(inputs["lin_w"]).T)
    w["pos_wT"] = np.ascontiguousarray(np.asarray(inputs["pos_w"]).T, f)
    w["th_hi"], w["th_lo"] = _split16(np.asarray(inputs["to_hidden_w"]).T)
    w["af_hi"], w["af_lo"] = _split16(np.asarray(inputs["att_from_w"]).T)
    w["at_hi"], w["at_lo"] = _split16(np.asarray(inputs["att_to_w"]).T)
    w["fh_hi"], w["fh_lo"] = _split16(np.asarray(inputs["from_hidden_w"]).T)
    w["ow_hi"], w["ow_lo"] = _split16(np.asarray(inputs["out_w"]).T)
    cw = np.asarray(inputs["conv_w"], f).reshape(NL, 2, 8, 128, H2, 3)
    cw = cw.transpose(0, 2, 4, 1, 5, 3)          # [l, pair, i, ab, k, o_p]
    cw = np.ascontiguousarray(cw.reshape(NL, 8, 8, 128, 2, 3, 128))
    wh = cw.astype(np.float16)
    wl = cw - wh.astype(f)
    w["cw_hi"] = np.ascontiguousarray(
        (wh.astype(f) * CS).astype(np.float16))
    w8 = np.empty((NL, 8, 8, 128, 2, 3, 2, 128), E4)
    w8[..., 0, :] = (wl * CS).astype(E4)
    w8[..., 1, :] = wh.astype(E4)
    w["cw8"] = np.ascontiguousarray(w8)
    for k in ("lin_b", "pos_b", "to_hidden_b", "conv_b", "att_from_b",
              "att_to_b", "from_hidden_b", "out_b"):
        w[k] = np.ascontiguousarray(inputs[k], f)
    return w


LAST_RES = None


def kernel(_trace=False, **inputs):
    global LAST_RES
    nc = _get_nc()
    w = host_prep(inputs)
    re_hi, re_lo = _split16(np.asarray(inputs["residual_encoder"]))
    in_maps = []
    for c in range(NCORES):
        sl = slice(B_LOC * c, B_LOC * (c + 1))
        m = dict(w)
        m["target"] = np.ascontiguousarray(inputs["target"][sl], np.float32)
        m["hidden_encoder"] = np.ascontiguousarray(
            inputs["hidden_encoder"][sl], np.float32)
        m["re_hi"] = np.ascontiguousarray(re_hi[sl])
        m["re_lo"] = np.ascontiguousarray(re_lo[sl])
        in_maps.append(m)
    if _trace:
        try:
            import antenv.axon_hooks  # noqa: F401
        except ImportError:
            _trace = False
    res = run_bass_kernel_spmd(nc, in_maps, core_ids=list(range(NCORES)),
                               trace=_trace)
    LAST_RES = res
    return np.concatenate([res.results[c]["out"] for c in range(NCORES)],
                          axis=0)
